# revision 1
# baseline (speedup 1.0000x reference)
"""Trainium2 Bass kernel for nn_CholecMetric (segment_reduce).

Per-core (1 clip per NeuronCore, data-parallel over N=8):
  score[h,w] = (sum_p iog_max[p] * Gp[p,h,w]) / (sum_p Gp[p,h,w])
  where iog_max[p] = max_t |Gp_p & Gt_t| / |Gt_t|   (0 where undefined)

The masks are 0/1, so the host pre-casts both inputs to fp8 (exact) and
pre-shuffles them into the exact SBUF layouts the kernel wants. Device
HBM traffic drops 4x (3.2MB/core) and every load is a contiguous HWDGE
transfer on its own queue - no cast DMAs, no descriptor-generation cost.

Layout: hw = k*512 + c, k in [0,128) on partitions, c in [0,512) free.
  gp sbuf [128, 33, 512] fp8, host-built with slot 32 = ones baked in.
  gt host-shuffled to the canonical DoubleRow weights layout
  [k, j, i, t] (j = c-pair, i = pair element, t contiguous).
  Intersections: 256 fp8 DoubleRow matmuls (effective K=256 each)
  accumulating psum[16,33]; col 32 = gt_area via the ones slot.
  cover: strided DVE tensor_reduce over p per c-range, overlapped with
  the matmul stream; rcov = 1/max(cover, 0.5). The last piece runs
  after the w-chain DVE ops to stay off the critical path.
  epilogue num = sum_p w[p]*Gp[p]: DVE STT chain | ACT prescale -> DVE
  bf16 adds | ACT prescale -> GpSimd adds; all SBUF-resident.
"""

import numpy as np
import ml_dtypes

import concourse.bass as bass
import concourse.bacc as bacc
import concourse.tile as tile
from concourse import mybir
from concourse.bass_utils import run_bass_kernel_spmd

N, P, T, H, W = 8, 32, 16, 256, 256
HW = H * W          # 65536
K, C = 128, 512     # hw = k*C + c
NCORES = 8

F32 = mybir.dt.float32
BF16 = mybir.dt.bfloat16
F8 = mybir.dt.float8e4
ALU = mybir.AluOpType
DR = mybir.MatmulPerfMode.DoubleRow

# c-chunk boundaries for the two parallel HWDGE streams
CHUNKS = (0, 128, 288, 512)
# cover-reduce ranges on DVE; the tail piece is emitted after the
# w-chain DVE ops (DVE queues are in-order)
COV_RANGES = ((0, 128), (128, 288), (288, 448))
COV_TAIL = (448, 512)

# epilogue p-split: DVE STT chain, ACT prescale -> DVE bf16 adds,
# ACT prescale -> GpSimd bf16 adds
NP_DVE = 12
NP_ACT = 8
NP_GPS = P - NP_DVE - NP_ACT


def build():
    nc = bacc.Bacc("TRN2", target_bir_lowering=False, debug=False,
                   num_devices=1)
    gp_d = nc.dram_tensor("gpf", [K, (P + 1) * C], F8, kind="ExternalInput")
    gt2_d = nc.dram_tensor("gt2f", [K, T * HW // K], F8,
                           kind="ExternalInput")
    id16_d = nc.dram_tensor("id16", [T, T], F32, kind="ExternalInput")
    id32_d = nc.dram_tensor("id32", [P, P], F32, kind="ExternalInput")
    out_d = nc.dram_tensor("score", [HW], F32, kind="ExternalOutput")

    gp_r = gp_d.rearrange("k (p c) -> k p c", c=C)    # [128, 33, 512]
    gt2_r = gt2_d.rearrange("k (j i t) -> k j i t", i=2, t=T)
    out_r = out_d.rearrange("(k c) -> k c", c=C)      # [128, 512]

    with tile.TileContext(nc) as tc:
        with (
            tc.tile_pool(name="main", bufs=1) as main,
            tc.tile_pool(name="psum", bufs=1, space="PSUM") as psum,
        ):
            gp_t = main.tile([K, P + 1, C], F8, tag="gp")
            gt2_t = main.tile([K, C // 2, 2, T], F8, tag="gt2")

            # two parallel HWDGE streams: gp on sync, gt2 on scalar
            for c0, c1 in zip(CHUNKS, CHUNKS[1:]):
                nc.scalar.dma_start(out=gt2_t[:, c0 // 2:c1 // 2, :, :],
                                    in_=gt2_r[:, c0 // 2:c1 // 2, :, :])
                nc.sync.dma_start(out=gp_t[:, :, c0:c1],
                                  in_=gp_r[:, :, c0:c1])

            # constants (tiny, after the payload dispatches)
            id16 = main.tile([T, T], F32, tag="id16")
            id32 = main.tile([P, P], F32, tag="id32")
            ones128 = main.tile([1, K], F32, tag="ones128")
            nc.scalar.dma_start(out=id16[:], in_=id16_d[:])
            nc.scalar.dma_start(out=id32[:], in_=id32_d[:])
            nc.vector.memset(ones128[:], 1.0)

            # intersections + gt_area: 256 DoubleRow matmuls (c-pairs)
            psum_i = psum.tile([T, P + 1], F32, tag="inters")
            gpv = gp_t[:, :, :]
            for j in range(C // 2):
                rhs = bass.AP(tensor=gpv.tensor, offset=gpv.offset + 2 * j,
                              ap=[gpv.ap[0], [1, 2], [C, P + 1]])
                nc.tensor.matmul(
                    psum_i[:], gt2_t[:, j, :, :], rhs,
                    start=(j == 0), stop=(j == C // 2 - 1), perf_mode=DR)

            # cover = sum_p Gp via strided DVE reduces per c-range;
            # rcov = 1/max(cover, 0.5) (exact for cover >= 1)
            covm = main.tile([K, C], F32, tag="covm")
            rcov = main.tile([K, C], F32, tag="rcov")

            def cover_piece(c0, c1):
                v = gp_t[:, 0:P, c0:c1]
                sap = bass.AP(tensor=v.tensor, offset=v.offset,
                              ap=[v.ap[0], [1, c1 - c0], [C, P]])
                nc.vector.tensor_reduce(covm[:, c0:c1], sap,
                                        mybir.AxisListType.X, ALU.add)
                nc.vector.tensor_scalar_max(covm[:, c0:c1],
                                            covm[:, c0:c1], 0.5)
                nc.vector.reciprocal(rcov[:, c0:c1], covm[:, c0:c1])

            for c0, c1 in COV_RANGES:
                cover_piece(c0, c1)

            # w-chain: iogs = inters/area, transpose, max_t, broadcast
            iog_all = main.tile([T, P + 1], F32, tag="iogall")
            nc.scalar.copy(iog_all[:], psum_i[:])
            rarea = main.tile([T, 1], F32, tag="rarea")
            nc.vector.tensor_scalar_max(rarea[:], iog_all[:, P:P + 1], 0.5)
            nc.vector.reciprocal(rarea[:], rarea[:])
            iogs = main.tile([T, P], F32, tag="iogs")
            nc.vector.tensor_scalar_mul(iogs[:], iog_all[:, 0:P],
                                        rarea[:, 0:1])
            psum_tr = psum.tile([P, T], F32, tag="tr")
            nc.tensor.transpose(psum_tr[:], iogs[:], id16[:])
            iomax = main.tile([P, 1], F32, tag="iomax")
            nc.vector.tensor_reduce(iomax[:], psum_tr[:],
                                    mybir.AxisListType.X, ALU.max)
            psum_wr = psum.tile([1, P], F32, tag="wr")
            nc.tensor.matmul(psum_wr[:], iomax[:], id32[:])
            w_row = main.tile([1, P], F32, tag="wrow")
            nc.scalar.copy(w_row[:], psum_wr[:])
            psum_wb = psum.tile([K, P], F32, tag="wb")
            nc.tensor.matmul(psum_wb[:], ones128[:], w_row[:])
            w_bc = main.tile([K, P], F32, tag="wbc")
            nc.vector.tensor_copy(w_bc[:], psum_wb[:])

            # cover tail (off the critical path: after the w-chain)
            cover_piece(*COV_TAIL)

            # num = sum_p w[p] * Gp[p], three chains over p
            acc_v = main.tile([K, C], F32, tag="accv")
            acc_a = main.tile([K, C], BF16, tag="acca")
            acc_g = main.tile([K, C], BF16, tag="accg")
            nc.vector.tensor_scalar_mul(acc_v[:], gp_t[:, 0, :],
                                        w_bc[:, 0:1])
            for p in range(1, NP_DVE):
                nc.vector.scalar_tensor_tensor(
                    acc_v[:], gp_t[:, p, :], w_bc[:, p:p + 1], acc_v[:],
                    ALU.mult, ALU.add)
            p0 = NP_DVE
            nc.scalar.mul(acc_a[:], gp_t[:, p0, :], w_bc[:, p0:p0 + 1])
            for p in range(p0 + 1, p0 + NP_ACT):
                at = main.tile([K, C], BF16, tag=f"at{p % 4}")
                nc.scalar.mul(at[:], gp_t[:, p, :], w_bc[:, p:p + 1])
                nc.vector.tensor_tensor(acc_a[:], acc_a[:], at[:], ALU.add)
            p0 = NP_DVE + NP_ACT
            nc.scalar.mul(acc_g[:], gp_t[:, p0, :], w_bc[:, p0:p0 + 1])
            for p in range(p0 + 1, P):
                gt_tmp = main.tile([K, C], BF16, tag=f"gat{p % 4}")
                nc.scalar.mul(gt_tmp[:], gp_t[:, p, :], w_bc[:, p:p + 1])
                nc.gpsimd.tensor_tensor(acc_g[:], acc_g[:], gt_tmp[:],
                                        ALU.add)

            # combine and divide
            nc.vector.tensor_tensor(acc_v[:], acc_v[:], acc_a[:], ALU.add)
            nc.vector.tensor_tensor(acc_v[:], acc_v[:], acc_g[:], ALU.add)
            score = main.tile([K, C], F32, tag="score")
            nc.vector.tensor_tensor(score[:], acc_v[:], rcov[:], ALU.mult)

            nc.sync.dma_start(out=out_r[:], in_=score[:])

    nc.compile()
    return nc


_NC_CACHE = None


def _get_nc():
    global _NC_CACHE
    if _NC_CACHE is None:
        _NC_CACHE = build()
    return _NC_CACHE


def kernel(groups_pred: np.ndarray, groups_true: np.ndarray, trace=False,
           **trace_kwargs) -> np.ndarray:
    nc = _get_nc()
    f8 = ml_dtypes.float8_e4m3
    gp = np.asarray(groups_pred, dtype=np.int32).reshape(N, P, K, C)
    gt = np.asarray(groups_true, dtype=np.int32).reshape(N, T, HW)
    # host pre-cast + pre-shuffle into the device layouts (masks are 0/1,
    # exact in fp8): gp -> [n, k, p, c] with a baked-in ones slot;
    # gt -> DoubleRow weights [n, k, j, i, t]
    gp_f = np.ones((N, K, P + 1, C), dtype=f8)
    gp_f[:, :, 0:P, :] = (gp != 0).transpose(0, 2, 1, 3).astype(f8)
    gp_f = gp_f.reshape(N, K, (P + 1) * C)
    gt2_f = np.ascontiguousarray(
        (gt != 0).reshape(N, T, K, C // 2, 2).transpose(0, 2, 3, 4, 1)
    ).astype(f8).reshape(N, K, T * HW // K)
    id16 = np.eye(T, dtype=np.float32)
    id32 = np.eye(P, dtype=np.float32)
    in_maps = [{"gpf": gp_f[n], "gt2f": gt2_f[n], "id16": id16,
                "id32": id32} for n in range(N)]
    res = run_bass_kernel_spmd(nc, in_maps, list(range(NCORES)), trace=trace,
                               **trace_kwargs)
    out = np.stack([res.results[n]["score"].reshape(H, W) for n in range(N)])
    if trace:
        kernel.last_results = res
    return out.astype(np.float32)



# revision 2
# speedup vs baseline: 1.0347x; 1.0347x over previous
"""Trainium2 Bass kernel for nn_CholecMetric (segment_reduce) — v2.

Per-core (1 clip per NeuronCore, data-parallel over N=8):
  score[hw] = (sum_p iog_max[p]*Gp[p,hw]) / (sum_p Gp[p,hw])
  iog_max[p] = max_t |Gp_p & Gt_t| / |Gt_t|

Layout: hw = k*512 + c, c = 8*g + 2*jj + i  (g in [0,64), jj in [0,4), i in [0,2))
Host pre-casts masks to fp8 (exact) and pre-shuffles:
  gp_m[k, g, (p,jj), i] fp8  — DoubleRow moving operand, p=32 slot = ones
  gt_w[k, g, i, (jj,t)] fp8  — DoubleRow weights
Intersections: 64 grouped DoubleRow matmuls (4 c-pairs per weight tile,
  m = jj*16+t weight cols, n = p*4+jj moving cols) accumulating
  psum[64, 132]; inters[t,p] = sum_jj psum[jj*16+t, p*4+jj].
Cover: wide pairwise-tree adds (5 ops per c-range; L1 fp8->bf16 uses both
  DVE read ports, upper levels bf16 2x) on DVE + GpSimd, chunk-staggered
  under the DMA/matmul stream. rcov = reciprocal_approx_fast(max(cov,.5)).
w-chain: 3 psum-strided adds -> rarea -> iogs -> PE transpose -> max ->
  2 broadcast matmuls.
num = sum_p w[p]*Gp[p]: balanced chains: DVE STT + ACT-mul->DVE-add +
  ACT-mul->GpSimd-add; combine/divide/out split in c-halves.
"""

import numpy as np
import ml_dtypes

import concourse.bass as bass
import concourse.bacc as bacc
import concourse.tile as tile
from concourse import mybir
from concourse.bass_utils import run_bass_kernel_spmd

N, P, T, H, W = 8, 32, 16, 256, 256
HW = H * W
K, C = 128, 512
G, JJ = 64, 4                 # c = 8g + 2jj + i
NM = (P + 1) * JJ             # 132 moving cols
MW = JJ * T                   # 64 weight cols (DR: out partitions 64)
NCORES = 8

F32 = mybir.dt.float32
BF16 = mybir.dt.bfloat16
F8 = mybir.dt.float8e4
ALU = mybir.AluOpType
DR = mybir.MatmulPerfMode.DoubleRow

# DMA chunk boundaries in g-units
CHUNKS = (0, 8, 16, 24, 32, 40, 48, 56, 64)

# cover tree ranges (g-units): GpSimd takes the earliest range,
# DVE the rest in land-order
COV_GPS = ((0, 14),)
COV_DVE = ((14, 40), (40, 64))

# num p-split: DVE-STT, ACT-mul->DVE-add, ACT-mul->GpSimd-add
NP_DVE_STT = 14
NP_ACT_DVE = 7
NP_ACT_GPS = P - NP_DVE_STT - NP_ACT_DVE   # 11


def _gp_slice(gp_t, p, g0, g1):
    """AP over gp_m for fixed p over g-range: [128, gspan, 8]."""
    gv = gp_t[:, :, :, :]
    return bass.AP(
        tensor=gv.tensor,
        offset=gv.offset + g0 * NM * 2 + p * JJ * 2,
        ap=[gv.ap[0], [NM * 2, g1 - g0], [1, JJ * 2]])


def _gp_pair(gp_t, p0, npair, g0, g1):
    """AP over gp_m: [128, gspan, npair, 8] selecting p = p0 + 2*q."""
    gv = gp_t[:, :, :, :]
    return bass.AP(
        tensor=gv.tensor,
        offset=gv.offset + g0 * NM * 2 + p0 * JJ * 2,
        ap=[gv.ap[0], [NM * 2, g1 - g0], [2 * JJ * 2, npair], [1, JJ * 2]])


def build():
    nc = bacc.Bacc("TRN2", target_bir_lowering=False, debug=False,
                   num_devices=1)
    gp_d = nc.dram_tensor("gpm", [K, G * NM * 2], F8, kind="ExternalInput")
    gt_d = nc.dram_tensor("gtw", [K, G * 2 * MW], F8, kind="ExternalInput")
    id16_d = nc.dram_tensor("id16", [T, T], F32, kind="ExternalInput")
    id32_d = nc.dram_tensor("id32", [P, P], F32, kind="ExternalInput")
    out_d = nc.dram_tensor("score", [HW], F32, kind="ExternalOutput")

    gp_r = gp_d.rearrange("k (g n i) -> k g n i", n=NM, i=2)
    gt_r = gt_d.rearrange("k (g i m) -> k g i m", i=2, m=MW)
    out_r = out_d.rearrange("(k c) -> k c", c=C)

    with tile.TileContext(nc) as tc:
        with (
            tc.tile_pool(name="main", bufs=1) as main,
            tc.tile_pool(name="psum", bufs=1, space="PSUM") as psum,
        ):
            gp_t = main.tile([K, G, NM, 2], F8, tag="gp")
            gt_t = main.tile([K, G, 2, MW], F8, tag="gt")

            # --- DMA: chunks on two HWDGE queues ---
            for c0, c1 in zip(CHUNKS, CHUNKS[1:]):
                nc.scalar.dma_start(out=gt_t[:, c0:c1, :, :],
                                    in_=gt_r[:, c0:c1, :, :])
                nc.sync.dma_start(out=gp_t[:, c0:c1, :, :],
                                  in_=gp_r[:, c0:c1, :, :])

            # constants (after payload dispatch)
            id16 = main.tile([T, T], F32, tag="id16")
            id32 = main.tile([P, P], F32, tag="id32")
            ones128 = main.tile([1, K], F32, tag="ones128")
            nc.scalar.dma_start(out=id16[:], in_=id16_d[:])
            nc.scalar.dma_start(out=id32[:], in_=id32_d[:])
            nc.vector.memset(ones128[:], 1.0)

            # --- intersections: 64 grouped DoubleRow matmuls ---
            psum_i = psum.tile([MW, NM], F32, tag="inters")
            for g in range(G):
                rhs = gp_t[:, g, :, :]
                rhs_ap = bass.AP(tensor=rhs.tensor, offset=rhs.offset,
                                 ap=[rhs.ap[0], [1, 2], [2, NM]])
                nc.tensor.matmul(psum_i[:], gt_t[:, g, :, :], rhs_ap,
                                 start=(g == 0), stop=(g == G - 1),
                                 perf_mode=DR)

            # --- cover trees (overlap with DMA/matmul stream) ---
            covm = main.tile([K, C], F32, tag="covm")
            rcov = main.tile([K, C], F32, tag="rcov")

            def cover_tree(eng, g0, g1, tag):
                gs = g1 - g0
                # L1: 16 pair-sums fp8+fp8 -> bf16 [k, gs, 16, 8]
                t1 = main.tile([K, gs, 16, 8], BF16, name=f"t1{tag}",
                               tag=f"t1{tag}")
                eng.tensor_tensor(t1[:], _gp_pair(gp_t, 0, 16, g0, g1),
                                  _gp_pair(gp_t, 1, 16, g0, g1), ALU.add)
                # L2..L4: bf16 pairwise halving
                prev, nq = t1, 16
                for lev in range(3):
                    nq //= 2
                    tn = main.tile([K, gs, nq, 8], BF16,
                                   name=f"t{lev+2}{tag}", tag=f"t{lev+2}{tag}")
                    pv = prev[:, :, :, :]
                    even = bass.AP(tensor=pv.tensor, offset=pv.offset,
                                   ap=[pv.ap[0], [nq * 16, gs], [16, nq],
                                       [1, 8]])
                    odd = bass.AP(tensor=pv.tensor, offset=pv.offset + 8,
                                  ap=[pv.ap[0], [nq * 16, gs], [16, nq],
                                      [1, 8]])
                    eng.tensor_tensor(tn[:], even, odd, ALU.add)
                    prev = tn
                # L5: final pair -> f32 covm range
                pv = prev[:, :, :, :]
                even = bass.AP(tensor=pv.tensor, offset=pv.offset,
                               ap=[pv.ap[0], [16, gs], [1, 8]])
                odd = bass.AP(tensor=pv.tensor, offset=pv.offset + 8,
                              ap=[pv.ap[0], [16, gs], [1, 8]])
                cv = covm[:, :]
                cout = bass.AP(tensor=cv.tensor, offset=cv.offset + g0 * 8,
                               ap=[cv.ap[0], [8, gs], [1, 8]])
                eng.tensor_tensor(cout, even, odd, ALU.add)

            for g0, g1 in COV_GPS:
                cover_tree(nc.gpsimd, g0, g1, f"g{g0}")
            for g0, g1 in COV_DVE[:2]:
                cover_tree(nc.vector, g0, g1, f"v{g0}")

            # --- w-chain ---
            # inters[t, p] = sum_jj psum_i[jj*16+t, p*4+jj]; the jj-blocks
            # live on different partition groups, so ACT-copy psum to SBUF,
            # DMA-move blocks 1..3 to partitions 0..15, then strided adds.
            s_all = main.tile([MW, NM], F32, tag="sall")
            nc.scalar.copy(s_all[:], psum_i[:])
            blks = main.tile([T, JJ - 1, NM], F32, tag="blks")
            nc.sync.dma_start(out=blks[:, 0, :], in_=s_all[T:2 * T, :])
            nc.scalar.dma_start(out=blks[:, 1, :], in_=s_all[2 * T:3 * T, :])
            nc.sync.dma_start(out=blks[:, 2, :], in_=s_all[3 * T:4 * T, :])

            iog_all = main.tile([T, P + 1], F32, tag="iogall")

            def blk0():
                pv = s_all[0:T, :]
                return bass.AP(tensor=pv.tensor, offset=pv.offset,
                               ap=[pv.ap[0], [JJ, P + 1]])

            def blk(jj):
                pv = blks[:, jj - 1, :]
                return bass.AP(tensor=pv.tensor, offset=pv.offset + jj,
                               ap=[pv.ap[0], [JJ, P + 1]])

            nc.vector.tensor_tensor(iog_all[:], blk0(), blk(1), ALU.add)
            nc.vector.tensor_tensor(iog_all[:], iog_all[:], blk(2), ALU.add)
            nc.vector.tensor_tensor(iog_all[:], iog_all[:], blk(3), ALU.add)

            rarea = main.tile([T, 1], F32, tag="rarea")
            nc.vector.tensor_scalar_max(rarea[:], iog_all[:, P:P + 1], 0.5)
            nc.vector.reciprocal_approx_fast(rarea[:], rarea[:])
            iogs = main.tile([T, P], F32, tag="iogs")
            nc.vector.tensor_scalar_mul(iogs[:], iog_all[:, 0:P],
                                        rarea[:, 0:1])
            psum_tr = psum.tile([P, T], F32, tag="tr")
            nc.tensor.transpose(psum_tr[:], iogs[:], id16[:])
            iomax = main.tile([P, 1], F32, tag="iomax")
            nc.vector.tensor_reduce(iomax[:], psum_tr[:],
                                    mybir.AxisListType.X, ALU.max)
            psum_wr = psum.tile([1, P], F32, tag="wr")
            nc.tensor.matmul(psum_wr[:], iomax[:], id32[:])
            w_row = main.tile([1, P], F32, tag="wrow")
            nc.scalar.copy(w_row[:], psum_wr[:])
            psum_wb = psum.tile([K, P], F32, tag="wb")
            nc.tensor.matmul(psum_wb[:], ones128[:], w_row[:])
            w_bc = main.tile([K, P], F32, tag="wbc")
            nc.scalar.copy(w_bc[:], psum_wb[:])

            # last cover tree + rcov (DVE) before the num chains
            for g0, g1 in COV_DVE[2:]:
                cover_tree(nc.vector, g0, g1, f"v{g0}")
            nc.vector.tensor_scalar_max(covm[:], covm[:], 0.5)
            nc.vector.reciprocal_approx_fast(rcov[:], covm[:])

            # --- num chains ---
            acc_d = main.tile([K, C], F32, tag="accd")
            acc_g = main.tile([K, C], BF16, tag="accg")

            dve_ps = list(range(0, NP_DVE_STT))
            ad_ps = list(range(NP_DVE_STT, NP_DVE_STT + NP_ACT_DVE))
            ag_ps = list(range(NP_DVE_STT + NP_ACT_DVE, P))

            tmp_ad = [main.tile([K, C], BF16, name=f"tad{i}", tag=f"tad{i}")
                      for i in (0, 1)]
            tmp_ag = [main.tile([K, C], BF16, name=f"tag{i}", tag=f"tag{i}")
                      for i in (0, 1)]

            # chain inits
            nc.scalar.mul(acc_g[:], _gp_slice(gp_t, ag_ps[0], 0, G),
                          w_bc[:, ag_ps[0]:ag_ps[0] + 1])
            nc.vector.tensor_scalar_mul(
                acc_d[:], _gp_slice(gp_t, dve_ps[0], 0, G),
                w_bc[:, dve_ps[0]:dve_ps[0] + 1])

            nsteps = max(len(ag_ps) - 1, len(ad_ps), len(dve_ps) - 1)
            for s in range(nsteps):
                if s < len(ag_ps) - 1:
                    p = ag_ps[s + 1]
                    nc.scalar.mul(tmp_ag[s % 2][:], _gp_slice(gp_t, p, 0, G),
                                  w_bc[:, p:p + 1])
                    nc.gpsimd.tensor_tensor(acc_g[:], acc_g[:],
                                            tmp_ag[s % 2][:], ALU.add)
                if s < len(ad_ps):
                    p = ad_ps[s]
                    nc.scalar.mul(tmp_ad[s % 2][:], _gp_slice(gp_t, p, 0, G),
                                  w_bc[:, p:p + 1])
                    nc.vector.tensor_tensor(acc_d[:], acc_d[:],
                                            tmp_ad[s % 2][:], ALU.add)
                if s < len(dve_ps) - 1:
                    p = dve_ps[s + 1]
                    nc.vector.scalar_tensor_tensor(
                        acc_d[:], _gp_slice(gp_t, p, 0, G),
                        w_bc[:, p:p + 1], acc_d[:], ALU.mult, ALU.add)

            # combine + divide + out, split in halves to overlap the DMA
            score = main.tile([K, C], F32, tag="score")
            for h0, h1 in ((0, C // 2), (C // 2, C)):
                nc.vector.tensor_tensor(acc_d[:, h0:h1], acc_d[:, h0:h1],
                                        acc_g[:, h0:h1], ALU.add)
                nc.vector.tensor_tensor(score[:, h0:h1], acc_d[:, h0:h1],
                                        rcov[:, h0:h1], ALU.mult)
                nc.sync.dma_start(out=out_r[:, h0:h1], in_=score[:, h0:h1])

    nc.compile()
    return nc


_NC_CACHE = None


def _get_nc():
    global _NC_CACHE
    if _NC_CACHE is None:
        _NC_CACHE = build()
    return _NC_CACHE


def _host_pack(groups_pred, groups_true):
    f8 = ml_dtypes.float8_e4m3
    gp = np.asarray(groups_pred, dtype=np.int32).reshape(N, P, K, G, JJ, 2)
    gt = np.asarray(groups_true, dtype=np.int32).reshape(N, T, K, G, JJ, 2)
    # gp_m[n, k, g, p, jj, i], ones at p=32
    gp_m = np.ones((N, K, G, P + 1, JJ, 2), dtype=f8)
    gp_m[:, :, :, :P] = (gp != 0).transpose(0, 2, 3, 1, 4, 5).astype(f8)
    gp_m = gp_m.reshape(N, K, G * NM * 2)
    # gt_w[n, k, g, i, jj, t]
    gt_w = np.ascontiguousarray(
        (gt != 0).transpose(0, 2, 3, 5, 4, 1)).astype(f8)
    gt_w = gt_w.reshape(N, K, G * 2 * MW)
    return gp_m, gt_w


def kernel(groups_pred: np.ndarray, groups_true: np.ndarray, trace=False,
           **trace_kwargs) -> np.ndarray:
    nc = _get_nc()
    gp_m, gt_w = _host_pack(groups_pred, groups_true)
    id16 = np.eye(T, dtype=np.float32)
    id32 = np.eye(P, dtype=np.float32)
    in_maps = [{"gpm": gp_m[n], "gtw": gt_w[n], "id16": id16, "id32": id32}
               for n in range(N)]
    res = run_bass_kernel_spmd(nc, in_maps, list(range(NCORES)), trace=trace,
                               **trace_kwargs)
    out = np.stack([res.results[n]["score"].reshape(H, W) for n in range(N)])
    if trace:
        kernel.last_results = res
    return out.astype(np.float32)
